# revision 16
# baseline (speedup 1.0000x reference)
"""Trainium2 Bass kernel for nn_Concat_73607149519362.

Math (decomposed concat-MLP attention score):
    score[b, d, e] = dec[b, d] @ w_dec + enc[b, e] @ w_enc + bias

Sharding: data-parallel over batch, 32 batches / 8 cores = 4 per core.

fp16 input/output DMA (fp32 accumulation; rel err ~3e-4 vs the 2e-2
gate) halves HBM traffic to ~17.3 MB/core. At fp16 all 4 batches fit in
SBUF at once, so every input DMA is issued eagerly with no WAR hazards.

Engine balance per batch (dec chunks first, then enc):
  DVE : 4 dec + 3 enc chunks via fused scalar_tensor_tensor (mult +
        accum, ~1.14us, no 2x mode) and 5 enc chunks as plain
        tensor_tensor mult (2x_1p fp16 mode, ~0.6us) whose accumulation
        runs on ACT. Last batch: 2 output builds.
  ACT : issues the two weight-broadcast DMAs on its HWDGE ring at t=0
        (fast ramp), then 5 accum-copies + enc_row + builds per batch.
  PE  : per-column transposes (interleaved with producers) flatten
        eproj into a (1, enc) PSUM row; 2 ones-outer-product matmuls
        rebroadcast the bias-folded row to (128, enc) PSUM.
  SP  : all enc/dec input DMAs issued eagerly (p-major contiguous runs).
  Pool: bias/ident/ones loads; output DMAs (last batch sliced/chunk).
"""

import os
from contextlib import ExitStack

os.environ.setdefault("JAX_PLATFORMS", "axon")

import numpy as np

import concourse.bass as bass
import concourse.mybir as mybir
from concourse.bass_utils import run_bass_kernel_spmd

B, DEC, ENC, DIM = 32, 512, 1024, 1024
NCORES = 8
BPC = B // NCORES  # batches per core

F32 = mybir.dt.float32
F16 = mybir.dt.float16
P = 128
NSPLIT = 5  # enc chunks whose accumulation runs on ACT
NSCRM = 3  # scratch slots for split-mult products


def _build(bpc=BPC, dec=DEC, enc=ENC, dim=DIM):
    nc = bass.Bass("TRN2")
    dec_h = nc.dram_tensor("dec_in", [bpc * dec, dim], F16, kind="ExternalInput")
    enc_h = nc.dram_tensor("enc_in", [bpc * enc, dim], F16, kind="ExternalInput")
    wenc_h = nc.dram_tensor("w_enc", [1, dim], F16, kind="ExternalInput")
    wdec_h = nc.dram_tensor("w_dec", [1, dim], F16, kind="ExternalInput")
    bias_h = nc.dram_tensor("bias", [1, 1], F32, kind="ExternalInput")
    ident_h = nc.dram_tensor("ident_in", [P, P], F32, kind="ExternalInput")
    ones_h = nc.dram_tensor("ones_in", [1, P], F32, kind="ExternalInput")
    out_h = nc.dram_tensor("out", [bpc * dec, enc], F16, kind="ExternalOutput")

    te = enc // P  # enc 128-row chunks per batch
    td = dec // P  # dec 128-row chunks per batch
    nblk = 512  # PSUM-bank-sized matmul block
    nh = enc // nblk
    last = bpc - 1

    dec_r = dec_h.ap().rearrange("(b p t) d -> b p t d", p=P, t=td)
    enc_r = enc_h.ap().rearrange("(b p t) d -> b p t d", p=P, t=te)
    out_r = out_h.ap().rearrange("(b p t) e -> b p t e", p=P, t=td)

    def enc_groups(b):
        if b == 0:
            return [(0, 2), (2, 4), (4, te)]
        if b == last:
            return [(0, 2), (2, 4), (4, 6), (6, te)]
        return [(0, te)]

    def dec_groups(b):
        if b == 0:
            return [(0, 2), (2, td)]
        return [(0, td)]

    # ---- schedules & semaphore count tables ----
    MUL = {}  # ("dec",b,t)/("enc",b,e)/("build",b,t) -> s_mult count after op
    ACC = {}  # ("accum",b,e)/("enc_row",b)/("build",b,t) -> s_acc count
    mc = ac = 0
    dve_sched = []
    act_sched = []
    for b in range(bpc):
        for t in range(td):
            mc += 1
            MUL[("dec", b, t)] = mc
            dve_sched.append(("dec", b, t))
        for e in range(te):
            mc += 1
            MUL[("enc", b, e)] = mc
            dve_sched.append(("enc", b, e))
        for e in range(NSPLIT):
            ac += 1
            ACC[("accum", b, e)] = ac
            act_sched.append(("accum", b, e))
        ac += 1
        ACC[("enc_row", b)] = ac
        act_sched.append(("enc_row", b, 0))
        for t in range(td):
            ac += 1
            ACC[("build", b, t)] = ac
            act_sched.append(("build", b, t))

    with ExitStack() as ctx:

        def sb(name, shape, dt=F32):
            return ctx.enter_context(nc.sbuf_tensor(name, shape, dt))

        w_enc_b = sb("w_enc_b", [P, dim], F16)
        w_dec_b = sb("w_dec_b", [P, dim], F16)
        bias_b = sb("bias_b", [P, 1])
        ident = sb("ident", [P, P])
        ones_row = sb("ones_row", [1, P])
        enc_t = [sb(f"enc_t{i}", [P, te, dim], F16) for i in range(bpc)]
        dec_t = [sb(f"dec_t{i}", [P, td, dim], F16) for i in range(bpc)]
        scr_f = sb("scr_f", [P, dim], F16)  # fused-op product dump
        scr_m = [sb(f"scr_m{i}", [P, dim], F16) for i in range(NSCRM)]
        eproj = [sb(f"eproj{i}", [P, te]) for i in range(bpc)]
        dproj = [sb(f"dproj{i}", [P, td]) for i in range(bpc)]
        enc_row = [sb(f"enc_row{i}", [1, enc]) for i in range(bpc)]
        out_t = [sb(f"out_t{i}", [P, td, enc], F16) for i in range(bpc)]
        tp_row = [
            ctx.enter_context(nc.psum_tensor(f"tp_row{i}", [1, enc], F32))
            for i in range(2)
        ]
        ebc = [
            ctx.enter_context(nc.psum_tensor(f"ebc{i}", [P, enc], F32))
            for i in range(2)
        ]

        s_we = ctx.enter_context(nc.semaphore(name="s_we"))
        s_wd = ctx.enter_context(nc.semaphore(name="s_wd"))
        s_misc = ctx.enter_context(nc.semaphore(name="s_misc"))
        s_enc = [
            [
                ctx.enter_context(nc.semaphore(name=f"s_enc{b}g{g}"))
                for g in range(len(enc_groups(b)))
            ]
            for b in range(bpc)
        ]
        s_dec = [
            [
                ctx.enter_context(nc.semaphore(name=f"s_dec{b}g{g}"))
                for g in range(len(dec_groups(b)))
            ]
            for b in range(bpc)
        ]
        s_mult = ctx.enter_context(nc.semaphore(name="s_mult"))
        s_acc = ctx.enter_context(nc.semaphore(name="s_acc"))
        s_pe = ctx.enter_context(nc.semaphore(name="s_pe"))
        s_out = ctx.enter_context(nc.semaphore(name="s_out"))

        with nc.Block(no_gpsimd_drain=True) as block:

            @block.sync
            def _(sync):
                for b in range(bpc):
                    for g, (lo, hi) in enumerate(dec_groups(b)):
                        sync.dma_start(
                            dec_t[b].ap()[:, lo:hi, :], dec_r[b][:, lo:hi, :]
                        ).then_inc(s_dec[b][g], 16)
                    for g, (lo, hi) in enumerate(enc_groups(b)):
                        sync.dma_start(
                            enc_t[b].ap()[:, lo:hi, :], enc_r[b][:, lo:hi, :]
                        ).then_inc(s_enc[b][g], 16)

            @block.gpsimd
            def _(gpsimd):
                gpsimd.dma_start(
                    w_dec_b.ap(), wdec_h.ap().to_broadcast((P, dim))
                ).then_inc(s_wd, 16)
                gpsimd.dma_start(
                    w_enc_b.ap(), wenc_h.ap().to_broadcast((P, dim))
                ).then_inc(s_we, 16)
                gpsimd.dma_start(
                    bias_b.ap(), bias_h.ap().to_broadcast((P, 1))
                ).then_inc(s_misc, 16)
                gpsimd.dma_start(ident.ap(), ident_h.ap()).then_inc(s_misc, 16)
                gpsimd.dma_start(ones_row.ap(), ones_h.ap()).then_inc(s_misc, 16)
                for b in range(bpc):
                    if b < last:
                        gpsimd.wait_ge(s_acc, ACC[("build", b, td - 1)])
                        nc.gpsimd.dma_start(out_r[b], out_t[b].ap()).then_inc(
                            s_out, 16
                        )
                    else:
                        for t in range(td):
                            gpsimd.wait_ge(s_acc, ACC[("build", b, t)])
                            nc.gpsimd.dma_start(
                                out_r[b][:, t, :], out_t[b].ap()[:, t, :]
                            ).then_inc(s_out, 16)

            @block.vector
            def _(vector):
                nmult = 0  # running index of split mults (for scr_m WAR)
                for kind, b, t in dve_sched:
                    if kind == "dec":
                        if b == 0 and t == 0:
                            vector.wait_ge(s_wd, 16)
                        for g, (lo, hi) in enumerate(dec_groups(b)):
                            if t == lo:
                                vector.wait_ge(s_dec[b][g], 16)
                        nc.vector.scalar_tensor_tensor(
                            out=scr_f.ap(),
                            in0=dec_t[b].ap()[:, t, :],
                            scalar=1.0,
                            in1=w_dec_b.ap(),
                            op0=mybir.AluOpType.mult,
                            op1=mybir.AluOpType.mult,
                            accum_out=dproj[b].ap()[:, t : t + 1],
                        ).then_inc(s_mult, 1)
                    elif kind == "enc":
                        if b == 0 and t == 0:
                            vector.wait_ge(s_we, 16)
                        for g, (lo, hi) in enumerate(enc_groups(b)):
                            if t == lo:
                                vector.wait_ge(s_enc[b][g], 16)
                        if t < NSPLIT:
                            # plain mult (2x fp16 mode); ACT accumulates
                            if nmult >= NSCRM:
                                pm = nmult - NSCRM
                                pb, pe = divmod(pm, NSPLIT)
                                vector.wait_ge(s_acc, ACC[("accum", pb, pe)])
                            nc.vector.tensor_tensor(
                                out=scr_m[nmult % NSCRM].ap(),
                                in0=enc_t[b].ap()[:, t, :],
                                in1=w_enc_b.ap(),
                                op=mybir.AluOpType.mult,
                            ).then_inc(s_mult, 1)
                            nmult += 1
                        else:
                            nc.vector.scalar_tensor_tensor(
                                out=scr_f.ap(),
                                in0=enc_t[b].ap()[:, t, :],
                                scalar=1.0,
                                in1=w_enc_b.ap(),
                                op0=mybir.AluOpType.mult,
                                op1=mybir.AluOpType.mult,
                                accum_out=eproj[b].ap()[:, t : t + 1],
                            ).then_inc(s_mult, 1)


            @block.tensor
            def _(pe):
                for b in range(bpc):
                    if b == 0:
                        pe.wait_ge(s_misc, 48)  # ident + ones ready
                    if b >= 2:
                        # tp_row slot free once b-2's enc_row add read it
                        pe.wait_ge(s_acc, ACC[("enc_row", b - 2)])
                    lasti = None
                    for t in range(te):
                        if t < NSPLIT:
                            pe.wait_ge(s_acc, ACC[("accum", b, t)])
                        else:
                            pe.wait_ge(s_mult, MUL[("enc", b, t)])
                        lasti = nc.tensor.transpose(
                            tp_row[b % 2].ap()[0:1, t * P : (t + 1) * P],
                            eproj[b].ap()[:, t : t + 1],
                            ident.ap(),
                        )
                    lasti.then_inc(s_pe, 1)
                    pe.wait_ge(s_acc, ACC[("enc_row", b)])
                    lasti = None
                    for h in range(nh):
                        lasti = nc.tensor.matmul(
                            ebc[b % 2].ap()[:, h * nblk : (h + 1) * nblk],
                            ones_row.ap(),
                            enc_row[b].ap()[0:1, h * nblk : (h + 1) * nblk],
                            start=True,
                            stop=True,
                        )
                    lasti.then_inc(s_pe, 1)

            @block.scalar
            def _(scalar):
                for kind, b, t in act_sched:
                    if kind == "accum":
                        scalar.wait_ge(s_mult, MUL[("enc", b, t)])
                        m = b * NSPLIT + t
                        nc.scalar.activation(
                            out=scr_m[m % NSCRM].ap(),
                            in_=scr_m[m % NSCRM].ap(),
                            func=mybir.ActivationFunctionType.Copy,
                            accum_out=eproj[b].ap()[:, t : t + 1],
                        ).then_inc(s_acc, 1)
                    elif kind == "enc_row":
                        if b == 0:
                            scalar.wait_ge(s_misc, 48)
                        scalar.wait_ge(s_pe, 2 * b + 1)
                        nc.scalar.add(
                            enc_row[b].ap().rearrange("o (p t) -> o p t", p=P),
                            tp_row[b % 2].ap().rearrange("o (t p) -> o p t", p=P),
                            add=bias_b.ap()[0:1, 0:1],
                        ).then_inc(s_acc, 1)
                    else:  # build
                        scalar.wait_ge(s_pe, 2 * b + 2)
                        scalar.wait_ge(s_mult, MUL[("dec", b, t)])
                        nc.scalar.add(
                            out_t[b].ap()[:, t, :],
                            ebc[b % 2].ap(),
                            add=dproj[b].ap()[:, t : t + 1],
                        ).then_inc(s_acc, 1)

    return nc


_NC_CACHE = {}


def _get_nc():
    if "nc" not in _NC_CACHE:
        _NC_CACHE["nc"] = _build()
    return _NC_CACHE["nc"]


_IDENT = np.eye(P, dtype=np.float32)
_ONES = np.ones((1, P), dtype=np.float32)


def _shard_inputs(decoder_states, encoder_states, mlp_weight, mlp_bias):
    decoder_states = np.asarray(decoder_states, dtype=np.float32).astype(np.float16)
    encoder_states = np.asarray(encoder_states, dtype=np.float32).astype(np.float16)
    decoder_states = np.ascontiguousarray(decoder_states)
    encoder_states = np.ascontiguousarray(encoder_states)
    mlp_weight = np.asarray(mlp_weight, dtype=np.float32).reshape(1, 2 * DIM)
    mlp_bias = np.ascontiguousarray(
        np.asarray(mlp_bias, dtype=np.float32).reshape(1, 1)
    )

    w_enc = np.ascontiguousarray(mlp_weight[:, :DIM].astype(np.float16))
    w_dec = np.ascontiguousarray(mlp_weight[:, DIM:].astype(np.float16))

    in_maps = []
    for i in range(NCORES):
        lo = i * BPC
        in_maps.append(
            {
                "dec_in": decoder_states[lo : lo + BPC].reshape(BPC * DEC, DIM),
                "enc_in": encoder_states[lo : lo + BPC].reshape(BPC * ENC, DIM),
                "w_enc": w_enc,
                "w_dec": w_dec,
                "bias": mlp_bias,
                "ident_in": _IDENT,
                "ones_in": _ONES,
            }
        )
    return in_maps


def _gather(res):
    shards = [
        r["out"].astype(np.float32).reshape(BPC, DEC, ENC) for r in res.results
    ]
    return np.concatenate(shards, axis=0)


def kernel(decoder_states, encoder_states, step, mlp_weight, mlp_bias, **_ignored):
    in_maps = _shard_inputs(decoder_states, encoder_states, mlp_weight, mlp_bias)
    res = run_bass_kernel_spmd(_get_nc(), in_maps, core_ids=list(range(NCORES)))
    return _gather(res)
